# revision 48
# baseline (speedup 1.0000x reference)
"""DynamicConv1D Trainium2 kernel.

Reference computation (per batch b):
  dw = conv1d(x, W, pad=3) + b            # [O*I*K, T] dynamic weights
  dw = softmax(dw.reshape(O,I,K,T)/sqrt(K), axis=K)
  y[o,t] = sum_{i,k} x[i, t+k-3] * dw[o,i,k,t]

Sharding: 8 cores = 4 batches x 2 halves of O (16 out-channels each).
Each core gets x[b] plus its half of the (rearranged) conv weights and
computes y[b, half*16:(half+1)*16, :]. No collectives; the host scatters
inputs and concatenates outputs.

Per-core layout (t-tile = 128 positions on partitions):
  conv as matmul: dw[t, (k,o,i)] = sum_{(j,c)} X1[(j,c), t] * W'[(j,c), (k,o,i)]
    X1[(j,c), u] = x[c, u+j-3]  (im2col layout built host-side, bf16)
    ones row appended to X1 so the bias rides as an extra W' row;
    1/sqrt(K) is folded into W' and b on the host.
  PE schedule: 8 chunks of 448 cols per tile, grouped 4+4 into two 4-bank
    PSUM groups that ping-pong; per group the stationary x1 block loads
    twice (a/b contraction halves) and 8 matmuls run back-to-back so the
    tensor engine stays at its high p-state.
  ScalarE drains each 4-bank group with one wide exp -> bf16 e in SBUF.
  DVE tail, all bf16 contiguous (2-elem-packed SBUF operands run 4x):
    ex = e * x2rep          (x2rep = x_unf replicated over o, shipped bf16)
    den/num = sum_k {e,ex}  (batched adds on the [TT,2,*] eex view)
    y[t,o] = sum_i num/den  (tensor_tensor divide + grouped reduce)
"""

import numpy as np

B = 4
C = 32
K = 7
T = 4096
O_FULL = 32
OH = 16  # out-channels per core
PAD = 3
TT = 128  # t positions per tile (partition dim)
FREE = K * OH * C  # 3584, free index = k*512 + o*32 + i
SLAB = OH * C  # 512, one k-slab
CD1 = 128  # (j, c) rows for j=0..3
CD2 = 97  # (j, c) rows for j=4..6 plus ones row
MCH = 448  # matmul chunk (cols); 8 per tile, 4 per psum group
GRP = 4 * MCH  # 1792 free elems drained per activation

_prog_cache = {}


def _register_div_op():
    """Register DIV_APPROX_ANT: out = in1/in0 via bit-trick seed + one NR pass.

    Same seed as RECIPROCAL_APPROX_FAST but only one Newton-Raphson
    iteration (err ~0.4%, on par with bf16) so the trailing multiply by
    Src1 fits the v3 stage budget. Works on bf16 operands (the DVE pipe
    upconverts to fp32 before the BITWISE_NOT exponent flip)."""
    import numpy as np
    from concourse import dve_ops
    from concourse.dve_spec import AluOp, Bin, Spec, Src0, Src1, C0, C1
    from concourse.dve_spec import lower, _has_src1
    from concourse.dve_uop import DveOpSpec

    name = "DIV_APPROX_ANT"
    for op in dve_ops.OPS:
        if op.name == name:
            return op

    _not_x = Bin(AluOp.BITWISE_NOT, Src0, Src0)
    _y0 = _not_x * C0
    _y1 = _y0 * (C1 - Src0 * _y0)

    def _ref(in0, in1, s0, s1, imm2):
        not_x = (~in0.astype(np.float32).view(np.int32)).view(np.float32)
        y0 = not_x * s0
        y1 = y0 * (s1 - in0 * y0)
        return (y1 * in1).astype(np.float32)

    spec = Spec(body=_y1 * Src1, reference=_ref)
    row = dve_ops._CUSTOM_DVE_ROW_BASE + len(dve_ops.OPS)
    shas = {}
    for ver in ("v3", "v4"):
        u = lower(spec, ver=ver)
        shas[ver] = DveOpSpec(
            name=name, opcode=row, uops=u, rd1_en=_has_src1(spec)
        ).sha(ver)
    op = dve_ops.DveOp(
        name, spec, subdim=False, uops_sha=shas,
        perf_en={"v3": True, "v4": True},
    )
    dve_ops.OPS.append(op)
    dve_ops._SUB_OPCODE_FOR_NAME[name] = row
    dve_ops.CUSTOM_DVE_SPECS[name] = spec
    return op


DIV_S0 = -0.23549792
DIV_S1 = 2.0017324


def _build(t_len):
    """Build and compile the per-core Bass program for sequence length t_len."""
    import concourse.tile as tile
    from concourse import bacc, mybir

    div_op = _register_div_op()
    nt = t_len // TT
    nc = bacc.Bacc("TRN2", target_bir_lowering=False, debug=False, num_devices=1)
    f32 = mybir.dt.float32
    bf16 = mybir.dt.bfloat16

    x1a_d = nc.dram_tensor("x1a", [CD1, t_len], bf16, kind="ExternalInput").ap()
    x1b_d = nc.dram_tensor("x1b", [CD2, t_len], bf16, kind="ExternalInput").ap()
    w1_d = nc.dram_tensor("wp1", [CD1, FREE], bf16, kind="ExternalInput").ap()
    w2_d = nc.dram_tensor("wp2", [CD2, FREE], bf16, kind="ExternalInput").ap()
    x2c_d = nc.dram_tensor("x2c", [TT, nt * K * C], bf16, kind="ExternalInput").ap()
    y_d = nc.dram_tensor("yout", [TT, nt * OH], bf16, kind="ExternalOutput").ap()

    with tile.TileContext(nc) as tc:
        with (
            tc.tile_pool(name="const", bufs=1) as cpool,
            tc.tile_pool(name="x2p", bufs=4) as x2pool,
            tc.tile_pool(name="ep", bufs=3) as epool,
            tc.tile_pool(name="tree", bufs=2) as tpool,
            tc.tile_pool(name="small", bufs=2) as spool,
            tc.tile_pool(name="pair", bufs=2) as qpool,
            tc.tile_pool(name="yout", bufs=2) as ypool,
            tc.tile_pool(name="psum", bufs=2, space="PSUM") as ppool,
        ):
            x1a_bf = cpool.tile([CD1, t_len], bf16, tag="x1abf")
            x1b_bf = cpool.tile([CD2, t_len], bf16, tag="x1bbf")
            w1_bf = cpool.tile([CD1, FREE], bf16, tag="w1bf")
            w2_bf = cpool.tile([CD2, FREE], bf16, tag="w2bf")

            # Constant loads, first-needed columns first so the PE can start
            # after ~1MB instead of after the full 3.5MB of constants.
            # The scalar queue is also hardware-DGE and idle at startup, so
            # the b-phase constants ride it instead of the gpsimd queue,
            # whose software-DGE completions land ~20us late.
            t0c = 8 * TT
            nc.sync.dma_start(x1a_bf[:, 0:t0c], x1a_d[:, 0:t0c])
            nc.scalar.dma_start(x1b_bf[:, 0:t0c], x1b_d[:, 0:t0c])
            nc.sync.dma_start(w1_bf[:, 0:GRP], w1_d[:, 0:GRP])
            nc.scalar.dma_start(w2_bf[:, 0:GRP], w2_d[:, 0:GRP])
            # Prefetch the first x_unf tiles ahead of the bulky weight
            # remainders so the DVE can start as soon as the first exp lands.
            x2c_pre = {}
            for pr in range(3):
                x2c_pre[pr] = x2pool.tile(
                    [TT, 2, K * C], bf16, tag="x2c", name="x2c"
                )
                nc.sync.dma_start(
                    x2c_pre[pr][:],
                    x2c_d[:, 2 * pr * K * C : 2 * (pr + 1) * K * C],
                )
            nc.sync.dma_start(w1_bf[:, GRP:], w1_d[:, GRP:])
            nc.scalar.dma_start(w2_bf[:, GRP:], w2_d[:, GRP:])
            nc.sync.dma_start(x1a_bf[:, t0c:], x1a_d[:, t0c:])
            nc.gpsimd.dma_start(x1b_bf[:, t0c:], x1b_d[:, t0c:])

            for tt in range(nt):
                t0 = tt * TT
                u = tt % 2
                if u == 0:
                    # compact x_unf for this tile pair, host-built: [TT,2,(k,i)]
                    pr = tt // 2
                    if pr in x2c_pre:
                        x2c2 = x2c_pre.pop(pr)
                    else:
                        x2c2 = x2pool.tile([TT, 2, K * C], bf16, tag="x2c")
                        nc.sync.dma_start(
                            x2c2[:],
                            x2c_d[:, tt * K * C : (tt + 2) * K * C],
                        )
                    # eex2[pair-half, {e,EX}, FREE]: e and EX side by side for
                    # a whole tile pair so every DVE op batches 2 tiles wide.
                    eex2 = epool.tile([TT, 2, 2, FREE], bf16, tag="eex2")

                for g in range(2):
                    pg = ppool.tile([TT, 4 * SLAB], f32, tag="pg", name="pg")
                    pgv = pg[:].rearrange("p (a b) -> p a b", a=4)
                    for c in range(4):
                        cs = slice((4 * g + c) * MCH, (4 * g + c + 1) * MCH)
                        nc.tensor.matmul(
                            pgv[:, c, 0:MCH], x1a_bf[:, t0 : t0 + TT],
                            w1_bf[:, cs], start=True, stop=False,
                        )
                    for c in range(4):
                        cs = slice((4 * g + c) * MCH, (4 * g + c + 1) * MCH)
                        nc.tensor.matmul(
                            pgv[:, c, 0:MCH], x1b_bf[:, t0 : t0 + TT],
                            w2_bf[:, cs], start=False, stop=True,
                        )
                    dst = eex2[:, u, 0, g * GRP : (g + 1) * GRP].rearrange(
                        "p (a b) -> p a b", a=4
                    )
                    nc.scalar.activation(
                        dst, pgv[:, :, 0:MCH], mybir.ActivationFunctionType.Exp
                    )

                if u == 0:
                    continue

                # EX = e * x_unf broadcast over o, for the whole pair.
                e4 = eex2[:, :, 0].rearrange(
                    "p u (k o i) -> p u k o i", k=K, o=OH
                )
                x24 = (
                    x2c2[:]
                    .rearrange("p u (k i) -> p u k i", k=K)
                    .unsqueeze(3)
                    .broadcast_to([TT, 2, K, OH, C])
                )
                ex4 = eex2[:, :, 1].rearrange(
                    "p u (k o i) -> p u k o i", k=K, o=OH
                )
                nc.vector.tensor_mul(ex4, e4, x24)

                # k-sum trees for den (over e) and num (over EX), batched as
                # one wide op per level over the pair via the [TT,2,2,*] view.
                # Slabs k0..k6; L1: (k0..2)+(k3..5), then L2a into dnp and the
                # (k6-folding) tail via a compute-DMA accumulate.
                ev = eex2[:]
                t1p = tpool.tile([TT, 2, 2, 3 * SLAB], bf16, tag="t1p")
                nc.vector.tensor_add(
                    t1p[:],
                    ev[:, :, :, 0 : 3 * SLAB],
                    ev[:, :, :, 3 * SLAB : 6 * SLAB],
                )
                if tt % 4 == 1:
                    dnp = qpool.tile([TT, 4, 2, SLAB], bf16, tag="dnp")
                j = (tt % 4) // 2
                nc.vector.tensor_add(
                    dnp[:, 2 * j : 2 * j + 2],
                    t1p[:, :, :, 0:SLAB],
                    t1p[:, :, :, SLAB : 2 * SLAB],
                )
                t2p = spool.tile([TT, 2, 2, SLAB], bf16, tag="t2p")
                nc.vector.tensor_add(
                    t2p[:], t1p[:, :, :, 2 * SLAB :], ev[:, :, :, 6 * SLAB :]
                )
                nc.gpsimd.dma_start(
                    dnp[:, 2 * j : 2 * j + 2], t2p[:],
                    accum_op=mybir.AluOpType.add,
                )

                if tt % 8 == 1:
                    y8 = ypool.tile([TT, 8 * OH], bf16, tag="y8")
                if tt % 4 == 3:
                    # y1 = num/den in one fused custom-DVE op, then the i-sum,
                    # batched over 4 tiles to amortize per-op overhead.
                    y1 = qpool.tile([TT, 4, SLAB], bf16, tag="y1")
                    nc.vector._custom_dve(
                        div_op,
                        out=y1[:],
                        in0=dnp[:, :, 0],
                        in1=dnp[:, :, 1],
                        s0=DIV_S0,
                        s1=DIV_S1,
                    )
                    q0 = (tt % 8 - 3) * OH
                    with nc.allow_low_precision(reason="y quantizes fine in bf16"):
                        nc.vector.tensor_reduce(
                            y8[:, q0 : q0 + 4 * OH],
                            y1[:].rearrange("p u (o i) -> p u o i", o=OH),
                            axis=mybir.AxisListType.X,
                            op=mybir.AluOpType.add,
                        )

                if (tt + 1) % 8 == 0 or tt == nt - 1:
                    g0 = (tt // 8) * 8 * OH
                    nc.sync.dma_start(
                        y_d[:, g0 : (tt + 1) * OH], y8[:, 0 : (tt + 1) * OH - g0]
                    )

    nc.compile()
    return nc


def _prep_inputs(x, W, b):
    """Host-side scatter: per-core input dicts (pure layout/slicing)."""
    import ml_dtypes

    bf = ml_dtypes.bfloat16
    scale = np.float32(1.0 / np.sqrt(K))
    halves = []
    for h in range(2):
        Wh = W[h * OH * C * K : (h + 1) * OH * C * K]  # [OH*C*K, C, K]
        # rows (j,c) -> j*32+c ; cols (k,o,i) -> k*512 + o*32 + i
        Wp = (
            Wh.reshape(OH, C, K, C, K).transpose(4, 3, 2, 0, 1).reshape(K * C, FREE)
            * scale
        )
        bh = (
            b[h * OH * C * K : (h + 1) * OH * C * K]
            .reshape(OH, C, K)
            .transpose(2, 0, 1)
            .reshape(FREE)
            * scale
        )
        w1 = np.ascontiguousarray(Wp[:CD1])
        w2 = np.ascontiguousarray(
            np.concatenate([Wp[CD1:], bh[None, :]], axis=0)
        )
        halves.append((w1.astype(bf), w2.astype(bf)))

    t_len = x.shape[-1]
    nt = t_len // TT
    x1s = []
    x2rs = []
    for bi in range(B):
        xp = np.zeros((C, t_len + 2 * PAD), dtype=np.float32)
        xp[:, PAD : PAD + t_len] = x[bi]
        x1a = np.empty((CD1, t_len), dtype=np.float32)
        x1b = np.empty((CD2, t_len), dtype=np.float32)
        for j in range(K):
            tgt, r0 = (x1a, j * C) if j < 4 else (x1b, (j - 4) * C)
            tgt[r0 : r0 + C] = xp[:, j : j + t_len]
        x1b[CD2 - 1] = 1.0
        x1s.append((x1a.astype(bf), x1b.astype(bf)))

        # compact x_unf: x2c[t, k, i] = xp[i, t+k], laid out so each t-tile
        # is one contiguous [TT, K*C] DMA.
        xpT = np.ascontiguousarray(xp.T).astype(bf)  # [t_len+6, C]
        win = np.lib.stride_tricks.sliding_window_view(xpT, K, axis=0)
        # win[t, i, k] = xpT[t+k, i]
        x2 = win.transpose(0, 2, 1).reshape(nt, TT, K * C)  # [t, k, i]
        x2rs.append(
            np.ascontiguousarray(
                x2.transpose(1, 0, 2).reshape(TT, nt * K * C)
            )
        )

    in_maps = []
    for core in range(8):
        bi, h = divmod(core, 2)
        w1, w2 = halves[h]
        x1a, x1b = x1s[bi]
        in_maps.append(
            {"x1a": x1a, "x1b": x1b, "wp1": w1, "wp2": w2, "x2c": x2rs[bi]}
        )
    return in_maps


def _assemble(results, t_len):
    """Gather per-core [TT, nt*OH] outputs into [B, O_FULL, t_len]."""
    nt = t_len // TT
    y = np.empty((B, O_FULL, t_len), dtype=np.float32)
    for core, res in enumerate(results):
        bi, h = divmod(core, 2)
        arr = res["yout"].astype(np.float32).reshape(TT, nt, OH)  # [tp, tt, o]
        y[bi, h * OH : (h + 1) * OH, :] = arr.transpose(2, 1, 0).reshape(OH, t_len)
    return y


def _run(x, W, b, trace=False, trace_cores=None):
    from concourse.bass_utils import run_bass_kernel_spmd
    from concourse.bass_interp import get_hw_module

    t_len = x.shape[-1]
    key = ("prog", t_len)
    if key not in _prog_cache:
        nc = _build(t_len)
        nc.m = get_hw_module(nc.m)
        _prog_cache[key] = nc
    nc = _prog_cache[key]

    in_maps = _prep_inputs(x, W, b)
    res = run_bass_kernel_spmd(
        nc,
        in_maps,
        core_ids=list(range(8)),
        trace=trace,
        trace_cores=trace_cores,
    )
    return _assemble(res.results, t_len), res


def kernel(x, W, b):
    y, _ = _run(np.asarray(x), np.asarray(W), np.asarray(b))
    return y


# revision 50
# speedup vs baseline: 1.1647x; 1.1647x over previous
"""DynamicConv1D Trainium2 kernel.

Reference computation (per batch b):
  dw = conv1d(x, W, pad=3) + b            # [O*I*K, T] dynamic weights
  dw = softmax(dw.reshape(O,I,K,T)/sqrt(K), axis=K)
  y[o,t] = sum_{i,k} x[i, t+k-3] * dw[o,i,k,t]

Sharding: 8 cores = 4 batches x 2 halves of O (16 out-channels each).
Each core gets x[b] plus its half of the (rearranged) conv weights and
computes y[b, half*16:(half+1)*16, :]. No collectives; the host scatters
inputs and concatenates outputs.

Per-core layout (t-tile = 128 positions on partitions):
  conv as matmul: dw[t, (k,o,i)] = sum_{(j,c)} X1[(j,c), t] * W'[(j,c), (k,o,i)]
    X1[(j,c), u] = x[c, u+j-3]  (im2col layout built host-side, bf16)
    ones row appended to X1 so the bias rides as an extra W' row;
    1/sqrt(K) is folded into W' and b on the host.
  PE schedule: 8 chunks of 448 cols per tile, grouped 4+4 into two 4-bank
    PSUM groups that ping-pong; per group the stationary x1 block loads
    twice (a/b contraction halves) and 8 matmuls run back-to-back so the
    tensor engine stays at its high p-state.
  ScalarE drains each 4-bank group with one wide exp -> bf16 e in SBUF.
  DVE tail, all bf16 contiguous (2-elem-packed SBUF operands run 4x):
    ex = e * x2rep          (x2rep = x_unf replicated over o, shipped bf16)
    den/num = sum_k {e,ex}  (batched adds on the [TT,2,*] eex view)
    y[t,o] = sum_i num/den  (tensor_tensor divide + grouped reduce)
"""

import numpy as np

B = 4
C = 32
K = 7
T = 4096
O_FULL = 32
OH = 16  # out-channels per core
PAD = 3
TT = 128  # t positions per tile (partition dim)
FREE = K * OH * C  # 3584, free index = k*512 + o*32 + i
SLAB = OH * C  # 512, one k-slab
CD1 = 128  # (j, c) rows for j=0..3
CD2 = 97  # (j, c) rows for j=4..6 plus ones row
MCH = 448  # matmul chunk (cols); 8 per tile, 4 per psum group
GRP = 4 * MCH  # 1792 free elems drained per activation

_prog_cache = {}


def _register_div_op():
    """Register DIV_APPROX_ANT: out = in1/in0 via bit-trick seed + one NR pass.

    Same seed as RECIPROCAL_APPROX_FAST but only one Newton-Raphson
    iteration (err ~0.4%, on par with bf16) so the trailing multiply by
    Src1 fits the v3 stage budget. Works on bf16 operands (the DVE pipe
    upconverts to fp32 before the BITWISE_NOT exponent flip)."""
    import numpy as np
    from concourse import dve_ops
    from concourse.dve_spec import AluOp, Bin, Spec, Src0, Src1, C0, C1
    from concourse.dve_spec import lower, _has_src1
    from concourse.dve_uop import DveOpSpec

    name = "DIV_APPROX_ANT"
    for op in dve_ops.OPS:
        if op.name == name:
            return op

    _not_x = Bin(AluOp.BITWISE_NOT, Src0, Src0)
    _y0 = _not_x * C0
    _y1 = _y0 * (C1 - Src0 * _y0)

    def _ref(in0, in1, s0, s1, imm2):
        not_x = (~in0.astype(np.float32).view(np.int32)).view(np.float32)
        y0 = not_x * s0
        y1 = y0 * (s1 - in0 * y0)
        return (y1 * in1).astype(np.float32)

    spec = Spec(body=_y1 * Src1, reference=_ref)
    row = dve_ops._CUSTOM_DVE_ROW_BASE + len(dve_ops.OPS)
    shas = {}
    for ver in ("v3", "v4"):
        u = lower(spec, ver=ver)
        shas[ver] = DveOpSpec(
            name=name, opcode=row, uops=u, rd1_en=_has_src1(spec)
        ).sha(ver)
    op = dve_ops.DveOp(
        name, spec, subdim=False, uops_sha=shas,
        perf_en={"v3": True, "v4": True},
    )
    dve_ops.OPS.append(op)
    dve_ops._SUB_OPCODE_FOR_NAME[name] = row
    dve_ops.CUSTOM_DVE_SPECS[name] = spec
    return op


DIV_S0 = -0.23549792
DIV_S1 = 2.0017324


def _build(t_len):
    """Build and compile the per-core Bass program for sequence length t_len."""
    import concourse.tile as tile
    from concourse import bacc, mybir

    div_op = _register_div_op()
    nt = t_len // TT
    nc = bacc.Bacc("TRN2", target_bir_lowering=False, debug=False, num_devices=1)
    f32 = mybir.dt.float32
    bf16 = mybir.dt.bfloat16

    x1a_d = nc.dram_tensor("x1a", [CD1, t_len], bf16, kind="ExternalInput").ap()
    x1b_d = nc.dram_tensor("x1b", [CD2, t_len], bf16, kind="ExternalInput").ap()
    w1_d = nc.dram_tensor("wp1", [CD1, FREE], bf16, kind="ExternalInput").ap()
    w2_d = nc.dram_tensor("wp2", [CD2, FREE], bf16, kind="ExternalInput").ap()
    x2c_d = nc.dram_tensor("x2c", [TT, nt * K * C], bf16, kind="ExternalInput").ap()
    y_d = nc.dram_tensor("yout", [TT, nt * OH], bf16, kind="ExternalOutput").ap()

    with tile.TileContext(nc) as tc:
        with (
            tc.tile_pool(name="const", bufs=1) as cpool,
            tc.tile_pool(name="x2p", bufs=4) as x2pool,
            tc.tile_pool(name="ep", bufs=3) as epool,
            tc.tile_pool(name="tree", bufs=2) as tpool,
            tc.tile_pool(name="small", bufs=2) as spool,
            tc.tile_pool(name="pair", bufs=2) as qpool,
            tc.tile_pool(name="yout", bufs=2) as ypool,
            tc.tile_pool(name="psum", bufs=2, space="PSUM") as ppool,
        ):
            x1a_bf = cpool.tile([CD1, t_len], bf16, tag="x1abf")
            x1b_bf = cpool.tile([CD2, t_len], bf16, tag="x1bbf")
            w1_bf = cpool.tile([CD1, FREE], bf16, tag="w1bf")
            w2_bf = cpool.tile([CD2, FREE], bf16, tag="w2bf")

            # Constant loads, first-needed columns first so the PE can start
            # after ~1MB instead of after the full 3.5MB of constants.
            t0c = 8 * TT
            nc.sync.dma_start(x1a_bf[:, 0:t0c], x1a_d[:, 0:t0c])
            nc.gpsimd.dma_start(x1b_bf[:, 0:t0c], x1b_d[:, 0:t0c])
            nc.sync.dma_start(w1_bf[:, 0:GRP], w1_d[:, 0:GRP])
            nc.gpsimd.dma_start(w2_bf[:, 0:GRP], w2_d[:, 0:GRP])
            # Prefetch the first x_unf tiles ahead of the bulky weight
            # remainders so the DVE can start as soon as the first exp lands.
            x2c_pre = {}
            for pr in range(3):
                x2c_pre[pr] = x2pool.tile(
                    [TT, 2, K * C], bf16, tag="x2c", name="x2c"
                )
                nc.sync.dma_start(
                    x2c_pre[pr][:],
                    x2c_d[:, 2 * pr * K * C : 2 * (pr + 1) * K * C],
                )
            nc.sync.dma_start(w1_bf[:, GRP:], w1_d[:, GRP:])
            nc.gpsimd.dma_start(w2_bf[:, GRP:], w2_d[:, GRP:])
            nc.sync.dma_start(x1a_bf[:, t0c:], x1a_d[:, t0c:])
            nc.gpsimd.dma_start(x1b_bf[:, t0c:], x1b_d[:, t0c:])

            for tt in range(nt):
                t0 = tt * TT
                u = tt % 2
                if u == 0:
                    # compact x_unf for this tile pair, host-built: [TT,2,(k,i)]
                    pr = tt // 2
                    if pr in x2c_pre:
                        x2c2 = x2c_pre.pop(pr)
                    else:
                        x2c2 = x2pool.tile([TT, 2, K * C], bf16, tag="x2c")
                        nc.sync.dma_start(
                            x2c2[:],
                            x2c_d[:, tt * K * C : (tt + 2) * K * C],
                        )
                    # eex2[pair-half, {e,EX}, FREE]: e and EX side by side for
                    # a whole tile pair so every DVE op batches 2 tiles wide.
                    eex2 = epool.tile([TT, 2, 2, FREE], bf16, tag="eex2")

                for g in range(2):
                    pg = ppool.tile([TT, 4 * SLAB], f32, tag="pg", name="pg")
                    pgv = pg[:].rearrange("p (a b) -> p a b", a=4)
                    for c in range(4):
                        cs = slice((4 * g + c) * MCH, (4 * g + c + 1) * MCH)
                        nc.tensor.matmul(
                            pgv[:, c, 0:MCH], x1a_bf[:, t0 : t0 + TT],
                            w1_bf[:, cs], start=True, stop=False,
                        )
                    for c in range(4):
                        cs = slice((4 * g + c) * MCH, (4 * g + c + 1) * MCH)
                        nc.tensor.matmul(
                            pgv[:, c, 0:MCH], x1b_bf[:, t0 : t0 + TT],
                            w2_bf[:, cs], start=False, stop=True,
                        )
                    dst = eex2[:, u, 0, g * GRP : (g + 1) * GRP].rearrange(
                        "p (a b) -> p a b", a=4
                    )
                    nc.scalar.activation(
                        dst, pgv[:, :, 0:MCH], mybir.ActivationFunctionType.Exp
                    )

                if u == 0:
                    continue

                # EX = e * x_unf broadcast over o, for the whole pair.
                e4 = eex2[:, :, 0].rearrange(
                    "p u (k o i) -> p u k o i", k=K, o=OH
                )
                x24 = (
                    x2c2[:]
                    .rearrange("p u (k i) -> p u k i", k=K)
                    .unsqueeze(3)
                    .broadcast_to([TT, 2, K, OH, C])
                )
                ex4 = eex2[:, :, 1].rearrange(
                    "p u (k o i) -> p u k o i", k=K, o=OH
                )
                nc.vector.tensor_mul(ex4, e4, x24)

                # k-sum trees for den (over e) and num (over EX), batched as
                # one wide op per level over the pair via the [TT,2,2,*] view.
                # Slabs k0..k6; L1: (k0..2)+(k3..5), then L2a into dnp and the
                # (k6-folding) tail via a compute-DMA accumulate.
                ev = eex2[:]
                t1p = tpool.tile([TT, 2, 2, 3 * SLAB], bf16, tag="t1p")
                nc.vector.tensor_add(
                    t1p[:],
                    ev[:, :, :, 0 : 3 * SLAB],
                    ev[:, :, :, 3 * SLAB : 6 * SLAB],
                )
                if tt % 4 == 1:
                    dnp = qpool.tile([TT, 4, 2, SLAB], bf16, tag="dnp")
                j = (tt % 4) // 2
                nc.vector.tensor_add(
                    dnp[:, 2 * j : 2 * j + 2],
                    t1p[:, :, :, 0:SLAB],
                    t1p[:, :, :, SLAB : 2 * SLAB],
                )
                t2p = spool.tile([TT, 2, 2, SLAB], bf16, tag="t2p")
                nc.vector.tensor_add(
                    t2p[:], t1p[:, :, :, 2 * SLAB :], ev[:, :, :, 6 * SLAB :]
                )
                nc.gpsimd.dma_start(
                    dnp[:, 2 * j : 2 * j + 2], t2p[:],
                    accum_op=mybir.AluOpType.add,
                )

                if tt % 8 == 1:
                    y8 = ypool.tile([TT, 8 * OH], bf16, tag="y8")
                if tt % 4 == 3:
                    # y1 = num/den in one fused custom-DVE op, then the i-sum,
                    # batched over 4 tiles to amortize per-op overhead.
                    y1 = qpool.tile([TT, 4, SLAB], bf16, tag="y1")
                    nc.vector._custom_dve(
                        div_op,
                        out=y1[:],
                        in0=dnp[:, :, 0],
                        in1=dnp[:, :, 1],
                        s0=DIV_S0,
                        s1=DIV_S1,
                    )
                    q0 = (tt % 8 - 3) * OH
                    with nc.allow_low_precision(reason="y quantizes fine in bf16"):
                        nc.vector.tensor_reduce(
                            y8[:, q0 : q0 + 4 * OH],
                            y1[:].rearrange("p u (o i) -> p u o i", o=OH),
                            axis=mybir.AxisListType.X,
                            op=mybir.AluOpType.add,
                        )

                if (tt + 1) % 8 == 0 or tt == nt - 1:
                    g0 = (tt // 8) * 8 * OH
                    nc.sync.dma_start(
                        y_d[:, g0 : (tt + 1) * OH], y8[:, 0 : (tt + 1) * OH - g0]
                    )

    nc.compile()
    return nc


def _prep_inputs(x, W, b):
    """Host-side scatter: per-core input dicts (pure layout/slicing)."""
    import ml_dtypes

    bf = ml_dtypes.bfloat16
    scale = np.float32(1.0 / np.sqrt(K))
    halves = []
    for h in range(2):
        Wh = W[h * OH * C * K : (h + 1) * OH * C * K]  # [OH*C*K, C, K]
        # rows (j,c) -> j*32+c ; cols (k,o,i) -> k*512 + o*32 + i
        Wp = (
            Wh.reshape(OH, C, K, C, K).transpose(4, 3, 2, 0, 1).reshape(K * C, FREE)
            * scale
        )
        bh = (
            b[h * OH * C * K : (h + 1) * OH * C * K]
            .reshape(OH, C, K)
            .transpose(2, 0, 1)
            .reshape(FREE)
            * scale
        )
        w1 = np.ascontiguousarray(Wp[:CD1])
        w2 = np.ascontiguousarray(
            np.concatenate([Wp[CD1:], bh[None, :]], axis=0)
        )
        halves.append((w1.astype(bf), w2.astype(bf)))

    t_len = x.shape[-1]
    nt = t_len // TT
    x1s = []
    x2rs = []
    for bi in range(B):
        xp = np.zeros((C, t_len + 2 * PAD), dtype=np.float32)
        xp[:, PAD : PAD + t_len] = x[bi]
        x1a = np.empty((CD1, t_len), dtype=np.float32)
        x1b = np.empty((CD2, t_len), dtype=np.float32)
        for j in range(K):
            tgt, r0 = (x1a, j * C) if j < 4 else (x1b, (j - 4) * C)
            tgt[r0 : r0 + C] = xp[:, j : j + t_len]
        x1b[CD2 - 1] = 1.0
        x1s.append((x1a.astype(bf), x1b.astype(bf)))

        # compact x_unf: x2c[t, k, i] = xp[i, t+k], laid out so each t-tile
        # is one contiguous [TT, K*C] DMA.
        xpT = np.ascontiguousarray(xp.T).astype(bf)  # [t_len+6, C]
        win = np.lib.stride_tricks.sliding_window_view(xpT, K, axis=0)
        # win[t, i, k] = xpT[t+k, i]
        x2 = win.transpose(0, 2, 1).reshape(nt, TT, K * C)  # [t, k, i]
        x2rs.append(
            np.ascontiguousarray(
                x2.transpose(1, 0, 2).reshape(TT, nt * K * C)
            )
        )

    in_maps = []
    for core in range(8):
        bi, h = divmod(core, 2)
        w1, w2 = halves[h]
        x1a, x1b = x1s[bi]
        in_maps.append(
            {"x1a": x1a, "x1b": x1b, "wp1": w1, "wp2": w2, "x2c": x2rs[bi]}
        )
    return in_maps


def _assemble(results, t_len):
    """Gather per-core [TT, nt*OH] outputs into [B, O_FULL, t_len]."""
    nt = t_len // TT
    y = np.empty((B, O_FULL, t_len), dtype=np.float32)
    for core, res in enumerate(results):
        bi, h = divmod(core, 2)
        arr = res["yout"].astype(np.float32).reshape(TT, nt, OH)  # [tp, tt, o]
        y[bi, h * OH : (h + 1) * OH, :] = arr.transpose(2, 1, 0).reshape(OH, t_len)
    return y


def _run(x, W, b, trace=False, trace_cores=None):
    from concourse.bass_utils import run_bass_kernel_spmd
    from concourse.bass_interp import get_hw_module

    t_len = x.shape[-1]
    key = ("prog", t_len)
    if key not in _prog_cache:
        nc = _build(t_len)
        nc.m = get_hw_module(nc.m)
        _prog_cache[key] = nc
    nc = _prog_cache[key]

    in_maps = _prep_inputs(x, W, b)
    res = run_bass_kernel_spmd(
        nc,
        in_maps,
        core_ids=list(range(8)),
        trace=trace,
        trace_cores=trace_cores,
    )
    return _assemble(res.results, t_len), res


def kernel(x, W, b):
    y, _ = _run(np.asarray(x), np.asarray(W), np.asarray(b))
    return y
